# revision 7
# baseline (speedup 1.0000x reference)
"""Trainium2 Bass kernel for DeepseekV2-style GQA prefill attention.

Self-contained: takes FULL inputs (as produced by the problem's
setup_inputs), shards tensor-parallel across 8 NeuronCores by heads
(4 q heads + 1 kv head per core), runs one SPMD Bass/Tile kernel, and
re-assembles the full outputs on host (partial-sum gather for o_proj,
scatter of K/V into the paged caches).

Per-core device program (all matmuls in float32r, 1 cycle/row):
  qkvT = w_qkv_local.T @ hiddenT          (transposed layout, [768, 2048])
  RoPE on qT/kT via partition half-swap (SBUF->SBUF DMA) +
    x*[cos;cos] + x_swapped*[-sin;sin]
  V = vT.T chunks via PE transpose
  per (head, q-supertile of 512): S^T = K @ qT chunks [128tk, 512tq],
    exp on ACT (scale folded in), causal mask-mul on diagonal chunks,
    denominator = ones-matmul over DVE-accumulated exp sums,
    attnT accumulated in PSUM, scaled by 1/denom -> DRAM scratch
  o_proj: outT_partial = w_o_local.T @ attnT   ([4096, 2048])
Host: out = (sum_c outT_partial_c).T ; caches scattered from kT/vT.
"""

import os
import sys

sys.path.insert(0, "/opt/trn_rl_repo")

import numpy as np

import concourse.bass as bass
import concourse.tile as tile
from concourse import bacc
from concourse import mybir
from concourse.bass_utils import run_bass_kernel_spmd

F32 = mybir.dt.float32
F32R = mybir.dt.float32r
AF = mybir.ActivationFunctionType

# Problem dims (hardcoded per contract)
T = 2048
H, KVH, D = 32, 8, 128
HID = H * D              # 4096
NCORES = 8
HPC = H // NCORES        # 4 q heads per core
JCOLS = HPC * D + 2 * D  # 768 local qkv columns (4 q heads + k + v)
NKC = HID // 128         # 32 contraction chunks for qkv
NST = 4                  # q supertiles
STW = 512                # supertile width
HW = 256                 # half-supertile width (qkv moving tile)
SCALE = float(D) ** -0.5

_PROGRAM = None


def _build_program():
    nc = bacc.Bacc("TRN2", target_bir_lowering=False, debug=False,
                   num_devices=NCORES)

    hT_d = nc.dram_tensor("hiddenT", [HID, T], F32R, kind="ExternalInput")
    wqkv_d = nc.dram_tensor("wqkv_local", [HID, JCOLS], F32R, kind="ExternalInput")
    wo_d = nc.dram_tensor("wo_local", [HPC * D, HID], F32R, kind="ExternalInput")
    cos2_d = nc.dram_tensor("cos2T", [128, T], F32, kind="ExternalInput")
    nsin2_d = nc.dram_tensor("nsin2T", [128, T], F32, kind="ExternalInput")
    mask_d = nc.dram_tensor("mask512", [512, 512], F32R, kind="ExternalInput")
    ident_d = nc.dram_tensor("ident128", [128, 128], F32R, kind="ExternalInput")
    onc_d = nc.dram_tensor("ones_col", [128, 1], F32R, kind="ExternalInput")
    onr_d = nc.dram_tensor("ones_row", [1, 128], F32R, kind="ExternalInput")

    outT_d = nc.dram_tensor("outT", [HID, T], F32, kind="ExternalOutput")
    kT_d = nc.dram_tensor("kT_out", [D, T], F32R, kind="ExternalOutput")
    vT_d = nc.dram_tensor("vT_out", [D, T], F32R, kind="ExternalOutput")

    attnT_d = nc.dram_tensor("attnT_scratch", [HPC * D, T], F32R)

    with nc.allow_low_precision(reason="float32r is 4-byte"), tile.TileContext(nc) as tc:
        with (
            tc.tile_pool(name="bigw", bufs=1) as bigw,
            tc.tile_pool(name="persist", bufs=1) as persist,
            tc.tile_pool(name="qt", bufs=2) as qtp,
            tc.tile_pool(name="work", bufs=2) as work,
            tc.tile_pool(name="psum", bufs=1, space="PSUM") as psum,
        ):
            # ---- resident tiles ----
            wqkv_sb = bigw.tile([128, NKC, JCOLS], F32R, tag="bigw", name="wqkv_sb")
            for c in range(NKC):
                nc.sync.dma_start(wqkv_sb[:, c, :], wqkv_d[128 * c:128 * c + 128, :])

            kT_sb = persist.tile([128, T], F32R, name="kT_sb")
            V_sb = persist.tile([128, 16, 128], F32R, name="V_sb")
            mask_sb = persist.tile([128, 4, 512], F32R, name="mask_sb")
            for jj in range(4):
                nc.sync.dma_start(mask_sb[:, jj, :], mask_d[128 * jj:128 * jj + 128, :])
            ident = persist.tile([128, 128], F32R, name="ident")
            nc.sync.dma_start(ident, ident_d[:, :])
            ones_c = persist.tile([128, 1], F32R, name="ones_c")
            nc.sync.dma_start(ones_c, onc_d[:, :])
            ones_r = persist.tile([1, 128], F32R, name="ones_r")
            nc.sync.dma_start(ones_r, onr_d[:, :])

            for st in range(NST):
                sts = slice(STW * st, STW * st + STW)

                # ---- qkv projection in half-supertiles ----
                for h2 in range(2):
                    off = STW * st + HW * h2
                    hs = slice(off, off + HW)
                    ls = slice(HW * h2, HW * h2 + HW)  # local within supertile

                    hT_sb = persist.tile([128, NKC, HW], F32R, tag="hT",
                                         name=f"hT_{st}_{h2}")
                    for c in range(NKC):
                        nc.sync.dma_start(hT_sb[:, c, :], hT_d[128 * c:128 * c + 128, hs])
                    cos_sb = work.tile([128, HW], F32, tag="cos", name=f"cos_{st}_{h2}")
                    sin_sb = work.tile([128, HW], F32, tag="sin", name=f"sin_{st}_{h2}")
                    nc.sync.dma_start(cos_sb, cos2_d[:, hs])
                    nc.sync.dma_start(sin_sb, nsin2_d[:, hs])

                    if h2 == 0:
                        qT_sb = qtp.tile([128, HPC, STW], F32R, tag="qt",
                                         name=f"qT_{st}")

                    for jb in range(6):
                        ps = psum.tile([128, HW], F32, tag="qkvps", bufs=2,
                                       name=f"qkvps_{st}_{h2}_{jb}")
                        for c in range(NKC):
                            nc.tensor.matmul(
                                ps,
                                wqkv_sb[:, c, 128 * jb:128 * jb + 128],
                                hT_sb[:, c, :],
                                start=(c == 0),
                                stop=(c == NKC - 1),
                            )
                        x_sb = work.tile([128, HW], F32R, tag="x",
                                         name=f"x_{st}_{h2}_{jb}")
                        nc.scalar.copy(x_sb, ps)
                        if jb < 5:
                            # RoPE: dst = x*[c;c] + swap(x)*[-s;s]
                            xsw = work.tile([128, HW], F32R, tag="xsw",
                                            name=f"xsw_{st}_{h2}_{jb}")
                            nc.sync.dma_start(xsw[0:64, :], x_sb[64:128, :])
                            nc.sync.dma_start(xsw[64:128, :], x_sb[0:64, :])
                            ta = work.tile([128, HW], F32, tag="ta",
                                           name=f"ta_{st}_{h2}_{jb}")
                            nc.vector.tensor_mul(ta, x_sb, cos_sb)
                            tb = work.tile([128, HW], F32, tag="tb",
                                           name=f"tb_{st}_{h2}_{jb}")
                            nc.vector.tensor_mul(tb, xsw, sin_sb)
                            dst = qT_sb[:, jb, ls] if jb < HPC else kT_sb[:, hs]
                            nc.vector.tensor_add(dst, ta, tb)
                        else:
                            # v bank: output vT and build V via PE transpose
                            nc.sync.dma_start(vT_d[:, hs], x_sb)
                            for i in range(2):
                                tp = psum.tile([128, 128], F32R, tag="tpps",
                                               name=f"tp_{st}_{h2}_{i}")
                                nc.tensor.transpose(
                                    tp, x_sb[:, 128 * i:128 * i + 128], ident)
                                nc.vector.tensor_copy(
                                    V_sb[:, 4 * st + 2 * h2 + i, :], tp)

                # ---- attention for this q supertile ----
                nch = 4 * (st + 1)
                for h in range(HPC):
                    av_ps = psum.tile([128, STW], F32, tag="avps", bufs=2,
                                      name=f"av_{st}_{h}")
                    acc = work.tile([128, STW], F32R, tag="acc", bufs=1,
                                    name=f"acc_{st}_{h}")
                    for jj in range(nch):
                        s_ps = psum.tile([128, STW], F32, tag="sps", bufs=2,
                                         name=f"s_{st}_{h}_{jj}")
                        nc.tensor.matmul(
                            s_ps,
                            kT_sb[:, 128 * jj:128 * jj + 128],
                            qT_sb[:, h, :],
                            start=True,
                            stop=True,
                        )
                        expT = work.tile([128, STW], F32R, tag="expT", bufs=2,
                                         name=f"e_{st}_{h}_{jj}")
                        nc.scalar.activation(expT, s_ps, AF.Exp, scale=SCALE)
                        if jj >= 4 * st:
                            nc.vector.tensor_mul(expT, expT,
                                                 mask_sb[:, jj - 4 * st, :])
                        if jj == 0:
                            nc.vector.tensor_copy(acc, expT)
                        else:
                            nc.vector.tensor_add(acc, acc, expT)
                        nc.tensor.matmul(
                            av_ps,
                            V_sb[:, jj, :],
                            expT,
                            start=(jj == 0),
                            stop=(jj == nch - 1),
                        )
                    den_ps = psum.tile([1, STW], F32, tag="dps",
                                       name=f"den_{st}_{h}")
                    nc.tensor.matmul(den_ps, ones_c,
                                     acc, start=True, stop=True)
                    recip = work.tile([1, STW], F32R, tag="recip", bufs=1,
                                      name=f"recip_{st}_{h}")
                    nc.vector.reciprocal(recip, den_ps)
                    bc_ps = psum.tile([128, STW], F32, tag="sps", bufs=2,
                                      name=f"bc_{st}_{h}")
                    nc.tensor.matmul(bc_ps, ones_r,
                                     recip, start=True, stop=True)
                    bc_sb = work.tile([128, STW], F32, tag="bcsb", bufs=1,
                                      name=f"bcsb_{st}_{h}")
                    nc.scalar.copy(bc_sb, bc_ps)
                    at_sb = work.tile([128, STW], F32R, tag="atsb", bufs=1,
                                      name=f"at_{st}_{h}")
                    nc.vector.tensor_mul(at_sb, av_ps, bc_sb)
                    nc.sync.dma_start(attnT_d[128 * h:128 * h + 128, sts], at_sb)

            nc.sync.dma_start(kT_d[:, :], kT_sb)

            # ---- o_proj ----
            wo_sb = bigw.tile([128, HPC, HID], F32R, tag="bigw", name="wo_sb")
            for c in range(HPC):
                nc.sync.dma_start(wo_sb[:, c, :], wo_d[128 * c:128 * c + 128, :])
            for tq in range(NST):
                tqs = slice(STW * tq, STW * tq + STW)
                stage = qtp.tile([128, HPC, STW], F32R, tag="qt", name=f"stage_{tq}")
                for c in range(HPC):
                    nc.sync.dma_start(stage[:, c, :],
                                      attnT_d[128 * c:128 * c + 128, tqs])
                for hob in range(HID // 128):
                    o_ps = psum.tile([128, STW], F32, tag="qkvps", bufs=2,
                                     name=f"o_{tq}_{hob}")
                    for c in range(HPC):
                        nc.tensor.matmul(
                            o_ps,
                            wo_sb[:, c, 128 * hob:128 * hob + 128],
                            stage[:, c, :],
                            start=(c == 0),
                            stop=(c == HPC - 1),
                        )
                    o_sb = work.tile([128, STW], F32, tag="osb", bufs=3,
                                     name=f"osb_{tq}_{hob}")
                    if hob % 2 == 0:
                        nc.vector.tensor_copy(o_sb, o_ps)
                    else:
                        nc.scalar.copy(o_sb, o_ps)
                    nc.sync.dma_start(outT_d[128 * hob:128 * hob + 128, tqs], o_sb)

    nc.compile()
    return nc


def _get_program():
    global _PROGRAM
    if _PROGRAM is None:
        _PROGRAM = _build_program()
    return _PROGRAM


LAST_RESULTS = None  # BassKernelResults of the most recent run (for profiling)


def kernel(hidden_states, cos, sin, w_qkv, w_o, k_cache, v_cache, slots):
    hidden_states = np.asarray(hidden_states, dtype=np.float32)
    cos = np.asarray(cos, dtype=np.float32)
    sin = np.asarray(sin, dtype=np.float32)
    w_qkv = np.asarray(w_qkv, dtype=np.float32)
    w_o = np.asarray(w_o, dtype=np.float32)
    slots = np.asarray(slots)

    hiddenT = np.ascontiguousarray(hidden_states.T)
    cosT = np.ascontiguousarray(cos.T)       # [64, T]
    sinT = np.ascontiguousarray(sin.T)
    cos2T = np.concatenate([cosT, cosT], axis=0)      # [128, T]
    nsin2T = np.concatenate([-sinT, sinT], axis=0)    # [128, T]
    # mask512[p, f] = 1.0 where p <= f (tk <= tq within diagonal supertile)
    mask512 = np.triu(np.ones((512, 512), np.float32))

    in_maps = []
    for c in range(NCORES):
        wq = w_qkv[:, 512 * c:512 * c + 512]
        wk = w_qkv[:, H * D + 128 * c: H * D + 128 * c + 128]
        wv = w_qkv[:, H * D + KVH * D + 128 * c: H * D + KVH * D + 128 * c + 128]
        wqkv_local = np.ascontiguousarray(np.concatenate([wq, wk, wv], axis=1))
        wo_local = np.ascontiguousarray(w_o[512 * c:512 * c + 512, :])
        in_maps.append({
            "hiddenT": hiddenT,
            "wqkv_local": wqkv_local,
            "wo_local": wo_local,
            "cos2T": cos2T,
            "nsin2T": nsin2T,
            "mask512": mask512,
            "ident128": np.eye(128, dtype=np.float32),
            "ones_col": np.ones((128, 1), np.float32),
            "ones_row": np.ones((1, 128), np.float32),
        })

    nc = _get_program()
    res = run_bass_kernel_spmd(nc, in_maps, list(range(NCORES)), trace=False)
    global LAST_RESULTS
    LAST_RESULTS = res

    outT = np.zeros((HID, T), np.float64)
    for r in res.results:
        outT += r["outT"].astype(np.float64)
    out = np.ascontiguousarray(outT.T.astype(np.float32))

    k_cache_new = np.array(k_cache, dtype=np.float32, copy=True)
    v_cache_new = np.array(v_cache, dtype=np.float32, copy=True)
    for c in range(NCORES):
        k_cache_new[slots, c, :] = res.results[c]["kT_out"].T
        v_cache_new[slots, c, :] = res.results[c]["vT_out"].T

    return out, k_cache_new, v_cache_new


# revision 26
# speedup vs baseline: 1.0495x; 1.0495x over previous
"""Trainium2 Bass kernel for DeepseekV2-style GQA prefill attention.

Self-contained: takes FULL inputs (as produced by the problem's
setup_inputs), shards tensor-parallel across 8 NeuronCores by heads
(4 q heads + 1 kv head per core), runs one SPMD Bass/Tile kernel, and
re-assembles the full outputs on host (partial-sum gather for o_proj,
scatter of K/V into the paged caches).

Per-core device program (all matmuls in float32r, 1 cycle/row):
  qkvT = w_qkv_local.T @ hiddenT          (transposed layout, [768, 2048])
  RoPE on qT/kT via partition half-swap (SBUF->SBUF DMA) +
    x*[cos;cos] + x_swapped*[-sin;sin]
  V = vT.T chunks via PE transpose
  per (head, q-supertile of 512): S^T = K @ qT chunks [128tk, 512tq],
    exp on ACT (scale folded in), causal mask-mul on diagonal chunks,
    denominator = ones-matmul over DVE-accumulated exp sums,
    attnT accumulated in PSUM, scaled by 1/denom -> DRAM scratch
  o_proj: outT_partial = w_o_local.T @ attnT   ([4096, 2048])
Host: out = (sum_c outT_partial_c).T ; caches scattered from kT/vT.
"""

import os
import sys

sys.path.insert(0, "/opt/trn_rl_repo")

import numpy as np

import concourse.bass as bass
import concourse.tile as tile
from concourse import bacc
from concourse import mybir
from concourse.bass_utils import run_bass_kernel_spmd

F32 = mybir.dt.float32
F32R = mybir.dt.float32r
AF = mybir.ActivationFunctionType

# Problem dims (hardcoded per contract)
T = 2048
H, KVH, D = 32, 8, 128
HID = H * D              # 4096
NCORES = 8
HPC = H // NCORES        # 4 q heads per core
JCOLS = HPC * D + 2 * D  # 768 local qkv columns (4 q heads + k + v)
NKC = HID // 128         # 32 contraction chunks for qkv
NST = 4                  # q supertiles
STW = 512                # supertile width
HW = 256                 # half-supertile width (qkv moving tile)
SCALE = float(D) ** -0.5

_PROGRAM = None


def _build_program():
    nc = bacc.Bacc("TRN2", target_bir_lowering=False, debug=False,
                   num_devices=NCORES)

    hT_d = nc.dram_tensor("hiddenT", [HID, T], F32R, kind="ExternalInput")
    wqkv_d = nc.dram_tensor("wqkv_local", [HID, JCOLS], F32R, kind="ExternalInput")
    wo_d = nc.dram_tensor("wo_local", [HPC * D, HID], F32R, kind="ExternalInput")
    cos2_d = nc.dram_tensor("cos2T", [128, T], F32, kind="ExternalInput")
    nsin2_d = nc.dram_tensor("nsin2T", [128, T], F32, kind="ExternalInput")
    mask_d = nc.dram_tensor("mask2", [128, 256], F32R, kind="ExternalInput")
    ident_d = nc.dram_tensor("ident128", [128, 128], F32R, kind="ExternalInput")
    pswap_d = nc.dram_tensor("pswap128", [128, 128], F32R, kind="ExternalInput")
    onc_d = nc.dram_tensor("ones_col", [128, 1], F32R, kind="ExternalInput")
    onr_d = nc.dram_tensor("ones_row", [1, 128], F32R, kind="ExternalInput")

    outT_d = nc.dram_tensor("outT", [HID, T], F32, kind="ExternalOutput")
    kT_d = nc.dram_tensor("kT_out", [D, T], F32R, kind="ExternalOutput")
    vT_d = nc.dram_tensor("vT_out", [D, T], F32R, kind="ExternalOutput")

    CB = 4           # hiddenT chunks per streamed batch tile
    NB = NKC // CB   # 8 batches per half-supertile

    with nc.allow_low_precision(reason="float32r is 4-byte"), \
            tile.TileContext(nc) as tc:
        with (
            tc.tile_pool(name="bigw", bufs=1) as bigw,
            tc.tile_pool(name="persist", bufs=1) as persist,
            tc.tile_pool(name="qt", bufs=2) as qtp,
            tc.tile_pool(name="work", bufs=2) as work,
            tc.tile_pool(name="psum", bufs=1, space="PSUM") as psum,
        ):
            # ---- resident tiles ----
            wqkv_r = wqkv_d[:, :].rearrange("(c p) j -> p c j", p=128)
            hT_r = hT_d[:, :].rearrange("(c p) t -> p c t", p=128)
            wqkv_sb = bigw.tile([128, NKC, JCOLS], F32R, tag="bigw", name="wqkv_sb")

            kT_sb = persist.tile([128, T], F32R, name="kT_sb")
            V_sb = persist.tile([128, 16, 128], F32R, name="V_sb")
            mask_sb = persist.tile([128, 256], F32R, name="mask_sb")
            nc.gpsimd.dma_start(mask_sb, mask_d[:, :])
            tri = mask_sb[:, 128:256]
            ident = persist.tile([128, 128], F32R, name="ident")
            nc.gpsimd.dma_start(ident, ident_d[:, :])
            pswap = persist.tile([128, 128], F32R, name="pswap")
            nc.gpsimd.dma_start(pswap, pswap_d[:, :])
            ones_c = persist.tile([128, 1], F32R, name="ones_c")
            nc.gpsimd.dma_start(ones_c, onc_d[:, :])
            ones_r = persist.tile([1, 128], F32R, name="ones_r")
            nc.gpsimd.dma_start(ones_r, onr_d[:, :])

            qts = {}
            stages = {}
            finished_heads = {}
            pending = []
            wo_holder = []

            def gen_qkv_half(st, h2):
                off = STW * st + HW * h2
                hs = slice(off, off + HW)
                ls = slice(HW * h2, HW * h2 + HW)  # local within supertile

                first = (st == 0 and h2 == 0)
                htcs = []
                for b in range(NB):
                    htc = persist.tile([128, CB, HW], F32R, tag="htc", bufs=4,
                                       name=f"htc_{st}_{h2}_{b}")
                    if first:
                        nc.sync.dma_start(
                            wqkv_sb[:, CB * b:CB * b + CB, :],
                            wqkv_r[:, CB * b:CB * b + CB, :])
                    nc.sync.dma_start(htc, hT_r[:, CB * b:CB * b + CB, hs])
                    htcs.append(htc)
                cos_sb = work.tile([128, HW], F32, tag="cos", bufs=1,
                                   name=f"cos_{st}_{h2}")
                sin_sb = work.tile([128, HW], F32, tag="sin", bufs=1,
                                   name=f"sin_{st}_{h2}")
                nc.gpsimd.dma_start(cos_sb, cos2_d[:, hs])
                nc.gpsimd.dma_start(sin_sb, nsin2_d[:, hs])

                if h2 == 0:
                    qts[st] = qtp.tile([128, HPC, STW], F32R, tag="qt",
                                       name=f"qT_{st}")
                qT_sb = qts[st]

                pbs = [psum.tile([128, 2, HW], F32, tag="qkvps", bufs=3,
                                 name=f"qkvps_{st}_{h2}_{j}") for j in range(3)]
                for b in range(NB):
                    for ci in range(CB):
                        c = CB * b + ci
                        for jb in range(6):
                            nc.tensor.matmul(
                                pbs[jb // 2][:, jb % 2, :],
                                wqkv_sb[:, c, 128 * jb:128 * jb + 128],
                                htcs[b][:, ci, :],
                                # start clears has_written for the WHOLE bank:
                                # only the very first matmul into each bank
                                # may set it; region B then sees cleared bits
                                # and overwrites on its first accumulation.
                                start=(c == 0 and jb % 2 == 0),
                                stop=(c == NKC - 1),
                                skip_group_check=True,
                            )
                        yield
                for jb in range(6):
                    ps = pbs[jb // 2][:, jb % 2, :]
                    x_sb = work.tile([128, HW], F32R, tag="x", bufs=1,
                                     name=f"x_{st}_{h2}_{jb}")
                    nc.scalar.copy(x_sb, ps)
                    if jb < 5:
                        sw_ps = psum.tile([128, HW], F32, tag="small",
                                          name=f"swps_{st}_{h2}_{jb}")
                        nc.tensor.matmul(sw_ps, pswap, x_sb,
                                         start=True, stop=True)
                        ta = work.tile([128, HW], F32, tag="ta", bufs=2,
                                       name=f"ta_{st}_{h2}_{jb}")
                        nc.vector.tensor_mul(ta, x_sb, cos_sb)
                        tb = work.tile([128, HW], F32, tag="tb", bufs=2,
                                       name=f"tb_{st}_{h2}_{jb}")
                        nc.vector.tensor_mul(tb, sw_ps, sin_sb)
                        dst = qT_sb[:, jb, ls] if jb < HPC else kT_sb[:, hs]
                        nc.vector.tensor_add(dst, ta, tb)
                    else:
                        nc.gpsimd.dma_start(vT_d[:, hs], x_sb)
                        for i in range(2):
                            tp = psum.tile([128, 128], F32R, tag="small",
                                           name=f"tp_{st}_{h2}_{i}")
                            nc.tensor.transpose(
                                tp, x_sb[:, 128 * i:128 * i + 128], ident)
                            nc.vector.tensor_copy(
                                V_sb[:, 4 * st + 2 * h2 + i, :], tp)
                    yield

            def gen_qkv(st):
                yield from gen_qkv_half(st, 0)
                yield from gen_qkv_half(st, 1)

            def emit_attn_finish():
                st, h, av_ps, acc = pending.pop()
                sts = slice(STW * st, STW * st + STW)
                den_ps = psum.tile([1, STW], F32, tag="small",
                                   name=f"den_{st}_{h}")
                nc.tensor.matmul(den_ps, ones_c, acc, start=True, stop=True)
                recip = work.tile([1, STW], F32R, tag="recip", bufs=1,
                                  name=f"recip_{st}_{h}")
                nc.vector.reciprocal(recip, den_ps)
                bc_ps = psum.tile([128, STW], F32, tag="sps", bufs=2,
                                  name=f"bc_{st}_{h}")
                nc.tensor.matmul(bc_ps, ones_r, recip, start=True, stop=True)
                bc_sb = work.tile([128, STW], F32, tag="bcsb", bufs=1,
                                  name=f"bcsb_{st}_{h}")
                nc.scalar.copy(bc_sb, bc_ps)
                at_sb = work.tile([128, STW], F32R, tag="atsb", bufs=16,
                                  name=f"at_{st}_{h}")
                nc.vector.tensor_mul(at_sb, av_ps, bc_sb)
                stages[(st, h)] = at_sb

            def gen_attn_head(st, h):
                qT_sb = qts[st]
                nch = 4 * (st + 1)
                av_ps = psum.tile([128, STW], F32, tag="avps", bufs=2,
                                  name=f"av_{st}_{h}")
                acc = work.tile([128, STW], F32R, tag="acc", bufs=2,
                                name=f"acc_{st}_{h}")
                exps = {}
                regions = {}

                def emit_s(jj):
                    jjl = jj - 4 * st
                    ns = 0 if jjl < 0 else min(128 * jjl, 256)
                    reg = slice(ns, STW)
                    regions[jj] = reg
                    s_ps = psum.tile([128, STW], F32, tag="sps", bufs=2,
                                     name=f"s_{st}_{h}_{jj}")
                    nc.tensor.matmul(
                        s_ps[:, reg],
                        kT_sb[:, 128 * jj:128 * jj + 128],
                        qT_sb[:, h, reg],
                        start=True,
                        stop=True,
                    )
                    expT = work.tile([128, STW], F32R, tag="expT", bufs=3,
                                     name=f"e_{st}_{h}_{jj}")
                    nc.scalar.activation(expT[:, reg], s_ps[:, reg],
                                         AF.Exp, scale=SCALE)
                    if jjl >= 0:
                        if jjl == 3:
                            nc.vector.tensor_mul(expT[:, 256:STW],
                                                 expT[:, 256:STW], mask_sb)
                        else:
                            ms = slice(128 * jjl, 128 * jjl + 128)
                            nc.vector.tensor_mul(expT[:, ms], expT[:, ms], tri)
                    if jj == 0:
                        nc.vector.tensor_copy(acc, expT)
                    else:
                        nc.vector.tensor_add(acc[:, reg], acc[:, reg],
                                             expT[:, reg])
                    exps[jj] = expT

                def emit_p(jj):
                    reg = regions.pop(jj)
                    nc.tensor.matmul(
                        av_ps[:, reg],
                        V_sb[:, jj, :],
                        exps.pop(jj)[:, reg],
                        start=(jj == 0),
                        stop=(jj == nch - 1),
                        skip_group_check=True,
                    )

                emit_s(0)
                yield
                emit_s(1)
                for jj in range(2, nch):
                    emit_s(jj)
                    emit_p(jj - 2)
                    yield
                emit_p(nch - 2)
                emit_p(nch - 1)
                if pending:
                    emit_attn_finish()
                yield
                pending.append((st, h, av_ps, acc))

            def gen_attn(st):
                for h in range(HPC):
                    yield from gen_attn_head(st, h)

            outT_r = outT_d[:, :].rearrange("(g i p) t -> g p i t", p=128, i=4)

            def gen_oproj(tq_list):
                wo_sb = wo_holder[0]
                for tq in tq_list:
                    tqs = slice(STW * tq, STW * tq + STW)
                    for g in range(8):
                        o_big = work.tile([128, 4, STW], F32, tag="osb", bufs=1,
                                          name=f"osb_{tq}_{g}")
                        for i in range(4):
                            hob = 4 * g + i
                            o_ps = psum.tile([128, STW], F32, tag="qkvps",
                                             bufs=3, name=f"o_{tq}_{hob}")
                            for c in range(HPC):
                                nc.tensor.matmul(
                                    o_ps,
                                    wo_sb[:, c, 128 * hob:128 * hob + 128],
                                    stages[(tq, c)],
                                    start=(c == 0),
                                    stop=(c == HPC - 1),
                                )
                            if hob % 2 == 0:
                                nc.vector.tensor_copy(o_big[:, i, :], o_ps)
                            else:
                                nc.scalar.copy(o_big[:, i, :], o_ps)
                            yield
                        nc.scalar.dma_start(outT_r[g][:, :, tqs], o_big)

            def drive(primary, filler):
                """Emit primary and filler interleaved, pacing filler by the
                remaining step ratio; drains both."""
                psteps = list(primary) if isinstance(primary, list) else None
                # count by running two passes is impossible (emission has side
                # effects) -- use a simple credit scheme instead
                raise RuntimeError

            def interleave(a, b, ratio):
                """Yield-drive generators a and b: per a-step, ~ratio b-steps."""
                credit = 0.0
                done_b = False
                for _ in a:
                    credit += ratio
                    while credit >= 1.0 and not done_b:
                        try:
                            next(b)
                        except StopIteration:
                            done_b = True
                        credit -= 1.0
                for _ in b:
                    pass

            # ---- pipeline driver ----
            for _ in gen_qkv(0):
                pass
            # qkv(st) has ~76 steps; attn(st-1) has 16*st + 4 steps
            for st in range(1, NST):
                a = gen_attn(st - 1)
                q = gen_qkv(st)
                interleave(a, q, ratio=(76.0 / (16 * st + 8)))
            # o_proj weights: transfer overlaps attention on supertile 3
            wo_sb = bigw.tile([128, HPC, HID], F32R, tag="bigw", name="wo_sb")
            wo_holder.append(wo_sb)
            wo_r = wo_d[:, :].rearrange("(c p) j -> p c j", p=128)
            for quarter in range(4):
                qsl = slice(HID // 4 * quarter, HID // 4 * (quarter + 1))
                nc.sync.dma_start(wo_sb[:, :, qsl], wo_r[:, :, qsl])
            # attn(3) interleaved with o_proj of supertiles 0..2
            interleave(gen_attn(NST - 1), gen_oproj([0, 1, 2]),
                       ratio=(96.0 / 68.0))
            while pending:
                emit_attn_finish()
            nc.gpsimd.dma_start(kT_d[:, :], kT_sb)
            for _ in gen_oproj([3]):
                pass

    nc.compile()
    return nc


def _get_program():
    global _PROGRAM
    if _PROGRAM is None:
        _PROGRAM = _build_program()
    return _PROGRAM


LAST_RESULTS = None  # BassKernelResults of the most recent run (for profiling)


def kernel(hidden_states, cos, sin, w_qkv, w_o, k_cache, v_cache, slots):
    hidden_states = np.asarray(hidden_states, dtype=np.float32)
    cos = np.asarray(cos, dtype=np.float32)
    sin = np.asarray(sin, dtype=np.float32)
    w_qkv = np.asarray(w_qkv, dtype=np.float32)
    w_o = np.asarray(w_o, dtype=np.float32)
    slots = np.asarray(slots)

    hiddenT = np.ascontiguousarray(hidden_states.T)
    cosT = np.ascontiguousarray(cos.T)       # [64, T]
    sinT = np.ascontiguousarray(sin.T)
    cos2T = np.concatenate([cosT, cosT], axis=0)      # [128, T]
    nsin2T = np.concatenate([-sinT, sinT], axis=0)    # [128, T]
    # mask2: [zeros | triangle(p<=f)] for diagonal-chunk masking
    mask2 = np.concatenate([np.zeros((128, 128), np.float32),
                            np.triu(np.ones((128, 128), np.float32))], axis=1)

    in_maps = []
    for c in range(NCORES):
        wq = w_qkv[:, 512 * c:512 * c + 512]
        wk = w_qkv[:, H * D + 128 * c: H * D + 128 * c + 128]
        wv = w_qkv[:, H * D + KVH * D + 128 * c: H * D + KVH * D + 128 * c + 128]
        wqkv_local = np.ascontiguousarray(np.concatenate([wq, wk, wv], axis=1))
        wo_local = np.ascontiguousarray(w_o[512 * c:512 * c + 512, :])
        in_maps.append({
            "hiddenT": hiddenT,
            "wqkv_local": wqkv_local,
            "wo_local": wo_local,
            "cos2T": cos2T,
            "nsin2T": nsin2T,
            "mask2": mask2,
            "ident128": np.eye(128, dtype=np.float32),
            "ones_col": np.ones((128, 1), np.float32),
            "ones_row": np.ones((1, 128), np.float32),
            "pswap128": np.roll(np.eye(128, dtype=np.float32), 64, axis=0),
        })

    nc = _get_program()
    res = run_bass_kernel_spmd(nc, in_maps, list(range(NCORES)), trace=False)
    global LAST_RESULTS
    LAST_RESULTS = res

    outT = np.zeros((HID, T), np.float64)
    for r in res.results:
        outT += r["outT"].astype(np.float64)
    out = np.ascontiguousarray(outT.T.astype(np.float32))

    k_cache_new = np.array(k_cache, dtype=np.float32, copy=True)
    v_cache_new = np.array(v_cache, dtype=np.float32, copy=True)
    for c in range(NCORES):
        k_cache_new[slots, c, :] = res.results[c]["kT_out"].T
        v_cache_new[slots, c, :] = res.results[c]["vT_out"].T

    return out, k_cache_new, v_cache_new
